# revision 6
# baseline (speedup 1.0000x reference)
"""LSTM cell (B=4096, D=U=2048) on 8 trn2 NeuronCores.

Tensor-parallel over units: core i computes units [i*256,(i+1)*256) of every
gate. Per core:
    z^T[1024 units, 4096 batch] = Wx_shard^T @ x^T + Wh_shard^T @ h^T
Mixed-precision matmuls, chosen per gate by error sensitivity (tanh g-gate
and o-gate dominate the final error, sigmoid f/i-gates are compressed 4x):
    g,o gates: bf16 matmuls (fp32 accumulate)
    f,i gates: fp8 e4m3 DoubleRow matmuls (2 k-planes/instr, 2x rate)
Gate activations fused with bias add + fp8 descale on ScalarE (units on
partitions -> bias is per-partition), elementwise LSTM combine on VectorE,
outputs stored transposed and re-transposed on the host.

fp8 scaling: x,h scaled by SX=4, f/i weights by SW=64; the f/i PSUM banks
hold 256*z and their activation applies scale=1/256 before the bias.
"""

import sys

sys.path.insert(0, "/opt/trn_rl_repo")

import ml_dtypes
import numpy as np

import concourse.bass as bass
import concourse.mybir as mybir
import concourse.tile as tile
from concourse.bass_utils import run_bass_kernel_spmd

B, D, U = 4096, 2048, 2048
N_CORES = 8
US = U // N_CORES          # units per core per gate (256)
UT = US // 128             # unit tiles of 128 per gate (2)
NB = 512                   # batch tile (free dim)
NT = B // NB               # batch tiles (8)
KX = D // 128              # k tiles for x gemm (16)
KH = U // 128              # k tiles for h gemm (16)
BF16 = mybir.dt.bfloat16
F8 = mybir.dt.float8e4
F32 = mybir.dt.float32
AF = mybir.ActivationFunctionType
DR = mybir.MatmulPerfMode.DoubleRow
SX = 4.0                   # activation fp8 scale
SW = 64.0                  # weight fp8 scale
SZ = 1.0 / (SX * SW)       # descale applied in the f/i gate activation

# gate index gi: 0=f 1=i 2=o 3=g.  f,i live in the fp8 weight tensors
# (local cols 0,1), o,g in the bf16 ones (local cols 0,1).
FP8_GATES = (0, 1)
LCOL = {0: 0, 1: 1, 2: 0, 3: 1}   # gi -> local gate slot in its dtype tensor


def _split_excess_waits(nc, maxw=1):
    """This walrus build rejects instructions carrying more than one sem-wait
    ("Too many sync wait commands"), but Tile freely attaches several. Hoist
    the extra waits onto same-engine nops inserted right before the
    instruction — engine streams are in-order, so blocking semantics are
    identical."""
    cnt = 0
    for fn in nc.m.functions:
        for bb in fn.blocks:
            new_insts = []
            for inst in bb.instructions:
                si = inst.sync_info
                waits = list(si.on_wait) if si is not None else []
                if len(waits) > maxw:
                    for i in range(0, len(waits) - maxw, maxw):
                        nop = mybir.InstNoOp(name=f"syncsplit-{cnt}")
                        cnt += 1
                        nop.engine = inst.engine
                        nop.sync_info = mybir.SyncInfo(
                            on_wait=waits[i : i + maxw], on_update=[]
                        )
                        new_insts.append(nop)
                    si.on_wait = waits[len(waits) - maxw :]
                new_insts.append(inst)
            if len(new_insts) != len(bb.instructions):
                bb.instructions = new_insts
    return cnt


def build_nc() -> bass.Bass:
    nc = bass.Bass()

    xTb = nc.dram_tensor("xTb", [D, B], BF16, kind="ExternalInput")
    hTb = nc.dram_tensor("hTb", [U, B], BF16, kind="ExternalInput")
    xT8 = nc.dram_tensor("xT8", [D, B], F8, kind="ExternalInput")
    hT8 = nc.dram_tensor("hT8", [U, B], F8, kind="ExternalInput")
    wxb = nc.dram_tensor("wxb", [D, 2 * US], BF16, kind="ExternalInput")  # [o,g]
    whb = nc.dram_tensor("whb", [U, 2 * US], BF16, kind="ExternalInput")
    wx8 = nc.dram_tensor("wx8", [D, 2 * US], F8, kind="ExternalInput")    # [f,i]
    wh8 = nc.dram_tensor("wh8", [U, 2 * US], F8, kind="ExternalInput")
    # bias, host-prepped to [128, 8]: column j = units [j*128,(j+1)*128) of
    # the concatenated [f,i,o,g] 1024-unit block (gate j//2, unit-tile j%2)
    bias = nc.dram_tensor("bias", [128, 4 * UT], F32, kind="ExternalInput")
    cT = nc.dram_tensor("cT", [US, B], F32, kind="ExternalInput")
    h_newT = nc.dram_tensor("h_newT", [US, B], F32, kind="ExternalOutput")
    c_newT = nc.dram_tensor("c_newT", [US, B], F32, kind="ExternalOutput")

    wxb_r = wxb.rearrange("(kt p) u -> p kt u", p=128)  # [128, KX, 512]
    whb_r = whb.rearrange("(kt p) u -> p kt u", p=128)
    wx8_r = wx8.rearrange("(kt p) u -> p kt u", p=128)
    wh8_r = wh8.rearrange("(kt p) u -> p kt u", p=128)
    xTb_r = xTb.rearrange("(kt p) b -> p kt b", p=128)  # [128, KX, B]
    hTb_r = hTb.rearrange("(kt p) b -> p kt b", p=128)
    xT8_r = xT8.rearrange("(kt p) b -> p kt b", p=128)
    hT8_r = hT8.rearrange("(kt p) b -> p kt b", p=128)

    with tile.TileContext(nc) as tc:
        with (
            tc.tile_pool(name="wpool", bufs=1) as wpool,
            tc.tile_pool(name="singles", bufs=1) as singles,
            tc.tile_pool(name="acts", bufs=2) as apool,
            tc.tile_pool(name="ew", bufs=2) as epool,
            tc.tile_pool(name="psum", bufs=8, space="PSUM") as ppool,
        ):
            ps_all = [
                [
                    ppool.tile([128, NB], F32, tag="ps", name=f"ps{ut}{gi}")
                    for gi in range(4)
                ]
                for ut in range(UT)
            ]

            # --- PE warmup: the HAM clock gate holds the PE at 1.2 GHz until
            # it has seen ~3.4us of sustained activity. Burn that window on
            # dummy matmuls while the startup DMA stream lands, so the real
            # matmuls run at 2.4 GHz from the first tile.
            dummy = singles.tile([128, NB], BF16)
            nc.vector.memset(dummy[:], 0.0)
            for w in range(8):
                nc.tensor.matmul(
                    ps_all[0][0][:],
                    dummy[:, :128],
                    dummy[:],
                    start=True,
                    stop=True,
                )

            # Startup on a single HWDGE ring (FIFO): bf16 x chunks + per-kt
            # bf16 weight tiles track the arrival stream for the g/o gates;
            # the fp8 side (only 1/3 of the PE work) loads as two whole tiles
            # slotted early in the stream.
            chunks = [(0, 2), (2, 4), (4, 8), (8, 12), (12, 16)]
            xb0 = {}   # kt -> [128, NB] bf16 AP
            hb0 = {}
            wxb_t = []
            whb_t = []
            nsl0 = bass.ts(0, NB)

            def startup_side(dst0, src_r, wsrc_r, w8src_r, wtiles, pre, x8tag):
                for ci, (k0, k1) in enumerate(chunks):
                    xc = apool.tile(
                        [128, k1 - k0, NB], BF16,
                        tag=f"{pre}c{k0}", bufs=1, name=f"{pre}c{k0}",
                    )
                    nc.sync.dma_start(out=xc[:], in_=src_r[:, k0:k1, nsl0])
                    for kt in range(k0, k1):
                        dst0[kt] = xc[:, kt - k0, :]
                    for kt in range(k0, k1):
                        wt = wpool.tile([128, 2 * US], BF16, tag=f"{pre}wb{kt}")
                        nc.scalar.dma_start(out=wt[:], in_=wsrc_r[:, kt, :])
                        wtiles.append(wt)
                    if ci == 0:
                        # fp8 activations for tile 0 go through the steady
                        # pool tag, in one DMA each
                        a8 = apool.tile([128, KX, NB], F8, tag=x8tag)
                        nc.sync.dma_start(out=a8[:], in_=_r8[:, :, nsl0])
                        w8 = wpool.tile([128, KX, 2 * US], F8, tag=f"{pre}w8full")
                        nc.scalar.dma_start(out=w8[:], in_=w8src_r[:, :, :])
                        startup_side.out8 = (a8, w8)
                return startup_side.out8

            _r8 = xT8_r
            x8_0, wx8_sb = startup_side(xb0, xTb_r, wxb_r, wx8_r, wxb_t, "x", "x_sb8")
            b_sb = singles.tile([128, 4 * UT], F32)
            nc.scalar.dma_start(out=b_sb[:], in_=bias[:])
            _r8 = hT8_r
            h8_0, wh8_sb = startup_side(hb0, hTb_r, whb_r, wh8_r, whb_t, "h", "h_sb8")

            # MM groups run in order [g, i, f, o]; each gate is consumed as
            # soon as possible so only o's short chain trails the last matmul
            GATE_ORDER = (3, 1, 0, 2)
            KP = KX // 2  # fp8 k-pairs per gemm (8)

            def act_gate(ps, gi, ut, name):
                g_sb = epool.tile([128, NB], F32, tag=f"gate{gi}", name=name)
                nc.scalar.activation(
                    g_sb[:],
                    ps[:],
                    AF.Tanh if gi == 3 else AF.Sigmoid,
                    bias=b_sb[:, gi * UT + ut : gi * UT + ut + 1],
                    scale=SZ if gi in FP8_GATES else 1.0,
                )
                return g_sb

            def elementwise(pss, n, ut):
                nsl = bass.ts(n, NB)
                usl = slice(ut * 128, (ut + 1) * 128)
                c_sb = epool.tile([128, NB], F32, tag="c_sb", name="c_sb")
                nc.gpsimd.dma_start(out=c_sb[:], in_=cT[usl, nsl])
                g_t = act_gate(pss[3], 3, ut, "g_t")
                i_t = act_gate(pss[1], 1, ut, "i_t")
                nc.vector.tensor_mul(i_t[:], i_t[:], g_t[:])      # i*g
                f_t = act_gate(pss[0], 0, ut, "f_t")
                nc.vector.tensor_mul(f_t[:], f_t[:], c_sb[:])     # f*c
                cn = epool.tile([128, NB], F32, tag="cn", name="cn")
                nc.vector.tensor_add(cn[:], f_t[:], i_t[:])       # c_new
                nc.gpsimd.dma_start(out=c_newT[usl, nsl], in_=cn[:])
                nc.scalar.activation(g_t[:], cn[:], AF.Tanh)      # tanh(c_new)
                o_t = act_gate(pss[2], 2, ut, "o_t")
                nc.vector.tensor_mul(o_t[:], o_t[:], g_t[:])      # h_new
                nc.gpsimd.dma_start(out=h_newT[usl, nsl], in_=o_t[:])

            def mm_bf16(ps, wt, act, gi, ut, start, stop):
                c0 = LCOL[gi] * US + ut * 128
                nc.tensor.matmul(
                    ps[:], wt[:, c0 : c0 + 128], act,
                    start=start, stop=stop,
                )

            def mm_fp8(ps, w8, act8, p, gi, ut, start, stop):
                c0 = LCOL[gi] * US + ut * 128
                nc.tensor.matmul(
                    ps[:],
                    w8[:, 2 * p : 2 * p + 2, c0 : c0 + 128],
                    act8[:, 2 * p : 2 * p + 2, :],
                    start=start, stop=stop, perf_mode=DR,
                )

            # --- n = 0: k-outer, tracking the startup arrival stream.
            # x side: g,o over kt 0,1; all f,i fp8 pairs; g,o over kt 2..15.
            # h side mirrors it; stop on the h side's last instrs per bank.
            for kt in (0, 1):
                for ut in range(UT):
                    for gi in (3, 2):
                        mm_bf16(ps_all[ut][gi], wxb_t[kt], xb0[kt], gi, ut,
                                kt == 0, False)
            for p in range(KP):
                for ut in range(UT):
                    for gi in (1, 0):
                        mm_fp8(ps_all[ut][gi], wx8_sb, x8_0, p, gi, ut,
                               p == 0, False)
            for kt in range(2, KX):
                for ut in range(UT):
                    for gi in (3, 2):
                        mm_bf16(ps_all[ut][gi], wxb_t[kt], xb0[kt], gi, ut,
                                False, False)
            for kt in (0, 1):
                for ut in range(UT):
                    for gi in (3, 2):
                        mm_bf16(ps_all[ut][gi], whb_t[kt], hb0[kt], gi, ut,
                                False, False)
            for p in range(KP):
                for ut in range(UT):
                    for gi in (1, 0):
                        mm_fp8(ps_all[ut][gi], wh8_sb, h8_0, p, gi, ut,
                               False, p == KP - 1)
            for kt in range(2, KH):
                for ut in range(UT):
                    for gi in (3, 2):
                        mm_bf16(ps_all[ut][gi], whb_t[kt], hb0[kt], gi, ut,
                                False, kt == KH - 1)
            for ut in range(UT):
                elementwise(ps_all[ut], 0, ut)

            # --- n = 1..7: gate-outer, k-inner; 4 groups in flight, the
            # other 4 banks cover the previous iteration's evacuation.
            for n in range(1, NT):
                nsl = bass.ts(n, NB)
                xb_sb = apool.tile([128, KX, NB], BF16, tag="xb_sb")
                nc.sync.dma_start(out=xb_sb[:], in_=xTb_r[:, :, nsl])
                x8_sb = apool.tile([128, KX, NB], F8, tag="x_sb8")
                nc.sync.dma_start(out=x8_sb[:], in_=xT8_r[:, :, nsl])
                hb_sb = apool.tile([128, KH, NB], BF16, tag="hb_sb")
                nc.sync.dma_start(out=hb_sb[:], in_=hTb_r[:, :, nsl])
                h8_sb = apool.tile([128, KH, NB], F8, tag="h_sb8")
                nc.sync.dma_start(out=h8_sb[:], in_=hT8_r[:, :, nsl])

                for ut in range(UT):
                    pss = [
                        ppool.tile([128, NB], F32, tag="ps", name=f"ps{gi}")
                        for gi in range(4)
                    ]
                    for gi in GATE_ORDER:
                        if gi in FP8_GATES:
                            for p in range(KP):
                                mm_fp8(pss[gi], wx8_sb, x8_sb, p, gi, ut,
                                       p == 0, False)
                            for p in range(KP):
                                mm_fp8(pss[gi], wh8_sb, h8_sb, p, gi, ut,
                                       False, p == KP - 1)
                        else:
                            for kt in range(KX):
                                mm_bf16(pss[gi], wxb_t[kt], xb_sb[:, kt, :],
                                        gi, ut, kt == 0, False)
                            for kt in range(KH):
                                mm_bf16(pss[gi], whb_t[kt], hb_sb[:, kt, :],
                                        gi, ut, False, kt == KH - 1)
                    elementwise(pss, n, ut)
    _split_excess_waits(nc)
    return nc


_NC_CACHE = None


def _get_nc():
    global _NC_CACHE
    if _NC_CACHE is None:
        _NC_CACHE = build_nc()
    return _NC_CACHE


def make_in_maps(x, h, c, Wxf, Wxi, Wxo, Wxg, bf, bi, bo, bg, Whf, Whi, Who, Whg):
    bf16 = ml_dtypes.bfloat16
    f8 = ml_dtypes.float8_e4m3
    xT = np.ascontiguousarray(np.asarray(x, np.float32).T)
    hT = np.ascontiguousarray(np.asarray(h, np.float32).T)
    xTb = xT.astype(bf16)
    hTb = hT.astype(bf16)
    xT8 = (xT * SX).astype(f8)
    hT8 = (hT * SX).astype(f8)
    c = np.asarray(c, np.float32)
    Wx = {k: np.asarray(w, np.float32) for k, w in
          zip("fiog", (Wxf, Wxi, Wxo, Wxg))}
    Wh = {k: np.asarray(w, np.float32) for k, w in
          zip("fiog", (Whf, Whi, Who, Whg))}
    bias = np.stack([np.asarray(v, np.float32) for v in (bf, bi, bo, bg)])

    in_maps = []
    for i in range(N_CORES):
        s = slice(i * US, (i + 1) * US)
        wxb_i = np.concatenate([Wx["o"][:, s], Wx["g"][:, s]], axis=1).astype(bf16)
        whb_i = np.concatenate([Wh["o"][:, s], Wh["g"][:, s]], axis=1).astype(bf16)
        wx8_i = (np.concatenate([Wx["f"][:, s], Wx["i"][:, s]], axis=1) * SW).astype(f8)
        wh8_i = (np.concatenate([Wh["f"][:, s], Wh["i"][:, s]], axis=1) * SW).astype(f8)
        b_i = np.concatenate([bias[g, s] for g in range(4)])  # [1024], f,i,o,g
        b_i = np.ascontiguousarray(b_i.reshape(4 * UT, 128).T)  # [128, 8]
        cT_i = np.ascontiguousarray(c[:, s].T)  # [US, B]
        in_maps.append(
            {
                "xTb": xTb, "hTb": hTb, "xT8": xT8, "hT8": hT8,
                "wxb": wxb_i, "whb": whb_i, "wx8": wx8_i, "wh8": wh8_i,
                "bias": b_i, "cT": cT_i,
            }
        )
    return in_maps


def run(in_maps, **kwargs):
    nc = _get_nc()
    return run_bass_kernel_spmd(nc, in_maps, list(range(N_CORES)), **kwargs)


def gather(results):
    h_new = np.empty((B, U), np.float32)
    c_new = np.empty((B, U), np.float32)
    for i in range(N_CORES):
        s = slice(i * US, (i + 1) * US)
        h_new[:, s] = results[i]["h_newT"].T
        c_new[:, s] = results[i]["c_newT"].T
    return h_new, c_new


def kernel(**inputs):
    res = run(make_in_maps(**inputs))
    return gather(res.results)
